# revision 1
# baseline (speedup 1.0000x reference)
"""Trainium2 Bass kernel for BaselineFeedforwardNetwork (dense_mlp).

Computation (per path n, step t):
    x_t   = [f_t (3), delta_{t-1} (1)]
    h     = relu(x_t @ W1 + b1)        # 4  -> 64
    h2    = relu(h @ W2 + b2)          # 64 -> 64
    delta = h2 @ W3 + b3               # 64 -> 1
Output: deltas (N, T).

Strategy (8 NeuronCores, pure data parallel over N):
  * hidden dim on SBUF partitions, paths on the free axis
  * per core: 32768 paths, processed as 8 passes of 4096 paths
    (8 chunks x 512); two passes run in lockstep ("lanes") so engines
    stay busy across the serial delta recurrence
  * all matmul operands bf16 (1 cyc/row on PE; fp32 would be 4x slower)
  * one in-place PSUM tile (4 banks) per lane per step: pre1 -> pre2 ->
    deltapre reuse the same banks (Tile serializes via true deps)
  * biases: per-partition bias APs on the activation ops; b3 immediate
  * delta chunk-select tricks: mm3 uses per-chunk lhsT columns so all 8
    chunks' deltas land on contiguous PSUM partitions 0..7; mm1b uses
    per-chunk lhsT rows to consume them from a partition-0-based tile
"""

import sys

for _p in ("/opt/trn_rl_repo",):
    if _p not in sys.path:
        sys.path.insert(0, _p)

import os
import numpy as np
import ml_dtypes

KLVL = int(os.environ.get("KLVL", "4"))  # debug: 1=mm1a/act1 2=+mm2/act2 3=+mm3/act3 4=full

NCORES = 8
N_TOT, T, FDIM = 262144, 60, 3
NC = N_TOT // NCORES          # 32768 paths per core
HID = 64
CH = 512                      # matmul free dim (one PSUM bank of fp32)
G = 8                         # chunks per pass-step
GP = G * CH                   # 4096 paths per pass
NPASS = NC // GP              # 8
NLANES = 2                    # passes in lockstep

# wpack column layout (all bf16, 128 partitions; every block duplicated on
# both partition halves so any chunk parity / lane can read it)
W1A_OFF = 0                                  # rows {0:3, 64:67} = W1[0:3]
M_OFF = 64                                   # rows 0:64 and 64:128 = W3 @ W1[3,:] (rank-1 fold)
W2_OFF = M_OFF + 64                          # rows 0:64 and 64:128 = W2
W3_OFF = W2_OFF + 64                         # [*, W3_OFF+32c+c] = W3 iff select col == c (dup halves)
WCOLS = W3_OFF + 32 * G                      # 448


def _build_graph(npass=NPASS, nsteps=T, b3val=0.0):
    import concourse.bacc as bacc
    from concourse import mybir
    from concourse.tile import TileContext

    BF = mybir.dt.bfloat16
    F32 = mybir.dt.float32

    import time as _time

    nc = bacc.Bacc(trn_type="TRN2", name=f"k{int(_time.time())}")

    feats_p = nc.declare_dram_parameter("feats", [T, FDIM, NC], BF, isOutput=False)
    wpack_p = nc.declare_dram_parameter("wpack", [128, WCOLS], BF, isOutput=False)
    bias_p = nc.declare_dram_parameter("biasp", [128, 4], F32, isOutput=False)
    out_p = nc.declare_dram_parameter("out", [T, NPASS * G, 2 * CH], BF, isOutput=True)

    with TileContext(nc) as tc:
        with (
            tc.tile_pool(name="consts", bufs=1) as cpool,
            tc.tile_pool(name="sbuf", bufs=2) as spool,
            tc.tile_pool(name="xqp", bufs=6) as xpool,
            tc.tile_pool(name="psum", bufs=1, space="PSUM") as ppool,
        ):
            wp = cpool.tile([128, WCOLS], BF, tag="wpack")
            bp = cpool.tile([128, 4], F32, tag="biasp")
            nc.sync.dma_start(out=wp[:, :], in_=wpack_p[:, :])
            nc.sync.dma_start(out=bp[:, :], in_=bias_p[:, :])

            # Warm-up: loads the ACT table + lets ACT/DVE observe const DMAs
            warm = cpool.tile([128, 4], F32, tag="warm")
            nc.scalar.activation(
                warm[:, 0:1], bp[:, 0:1],
                mybir.ActivationFunctionType.Relu, bias=0.0, scale=1.0,
            )
            nc.vector.tensor_scalar(
                warm[:, 1:2], bp[:, 1:2], 0.0, None, mybir.AluOpType.add,
            )

            def dma_x(ln, p, t0):
                xt = xpool.tile([67, GP], BF, tag=f"xq{ln}")
                nc.sync.dma_start(
                    out=xt[0:FDIM, :], in_=feats_p[t0, :, p * GP : (p + 1) * GP]
                )
                nc.sync.dma_start(
                    out=xt[64 : 64 + FDIM, :],
                    in_=feats_p[t0, :, p * GP : (p + 1) * GP],
                )
                return xt

            XPRE = 5  # steps of feature prefetch
            for ppair in range(npass // NLANES):
                lanes = [ppair * NLANES + ln for ln in range(NLANES)]
                xq = [[dma_x(ln, p, t0) for t0 in range(min(XPRE, nsteps))]
                      for ln, p in enumerate(lanes)]
                h2prev = [None] * NLANES
                for t in range(nsteps):
                    for ln, p in enumerate(lanes):
                        if t + XPRE < nsteps:
                            xq[ln].append(dma_x(ln, p, t + XPRE))
                        x = xq[ln][t]
                        P = ppool.tile([128, G // 2 * CH], F32, tag=f"pp{ln}")
                        h = spool.tile([128, G // 2 * CH], BF, tag=f"h{ln}")
                        h2 = spool.tile([128, G // 2 * CH], BF, tag=f"h2{ln}")
                        d_new = spool.tile([8, 2 * CH], BF, tag=f"d{ln}")

                        # lane-dependent partition parities: lane 0 uses the
                        # diagonal PE quadrants, lane 1 the anti-diagonal, so
                        # the two lanes' matmuls run on disjoint subarrays
                        def pH(c):   # pre1 / h partitions
                            return 64 * ((c % 2) ^ ln)

                        def pH2(c):  # pre2 / h2 partitions (and x row copy)
                            return 64 * (c % 2)

                        # ---- layer 1: pre1 = M^T h2prev (+ W1a^T f) ----
                        for c in range(G):
                            blk = (c // 2) * CH
                            o = P[pH(c) : pH(c) + HID, blk : blk + CH]
                            tp = (pH2(c), pH(c))
                            if t > 0:
                                nc.tensor.matmul(
                                    o,
                                    wp[pH2(c) : pH2(c) + HID, M_OFF : M_OFF + HID],
                                    h2prev[ln][pH2(c) : pH2(c) + HID, blk : blk + CH],
                                    start=True,
                                    stop=False,
                                    tile_position=tp,
                                )
                            nc.tensor.matmul(
                                o,
                                wp[pH2(c) : pH2(c) + FDIM, W1A_OFF : W1A_OFF + HID],
                                x[pH2(c) : pH2(c) + FDIM, c * CH : (c + 1) * CH],
                                start=(t == 0),
                                stop=True,
                                tile_position=tp,
                            )
                        # ---- act1: h = relu(pre1 + b1') ----
                        # t=0 uses plain b1 (no delta yet); t>0 uses
                        # b1' = b1 + W1[3,:]*b3 (completes the rank-1 fold)
                        nc.scalar.activation(
                            h[:, :], P[:, :],
                            mybir.ActivationFunctionType.Relu,
                            bias=bp[:, 3:4] if t == 0 else bp[:, 0:1],
                            scale=1.0,
                        )
                        # ---- layer 2 ----
                        for c in range(G):
                            blk = (c // 2) * CH
                            nc.tensor.matmul(
                                P[pH2(c) : pH2(c) + HID, blk : blk + CH],
                                wp[pH(c) : pH(c) + HID, W2_OFF : W2_OFF + HID],
                                h[pH(c) : pH(c) + HID, blk : blk + CH],
                                start=True,
                                stop=True,
                                tile_position=(pH(c), pH2(c)),
                            )
                        # ---- act2: h2 = relu(pre2 + b2) on DVE ----
                        nc.vector.tensor_scalar(
                            h2[:, :], P[:, :],
                            bp[:, 1:2], 0.0,
                            mybir.AluOpType.add, mybir.AluOpType.max,
                        )
                        # ---- layer 3 select: chunk deltas -> PSUM rows ----
                        # lane0 rows 0:8, lane1 rows 32:40; even chunks into
                        # cols 0:CH, odd into CH:2CH; uniform positions per group
                        dr = 32 * ln
                        for par in range(2):
                            cs = [c for c in range(G) if c % 2 == par]
                            for i, c in enumerate(cs):
                                blk = (c // 2) * CH
                                nc.tensor.matmul(
                                    P[dr : dr + 32, par * CH : (par + 1) * CH],
                                    wp[pH2(c) : pH2(c) + HID, W3_OFF + 32 * c : W3_OFF + 32 * (c + 1)],
                                    h2[pH2(c) : pH2(c) + HID, blk : blk + CH],
                                    start=(i == 0),
                                    stop=(i == len(cs) - 1),
                                    tile_position=(pH2(c), dr),
                                )
                        # ---- act3: delta = deltapre + b3 ----
                        nc.scalar.activation(
                            d_new[0:G, :], P[dr : dr + G, 0 : 2 * CH],
                            mybir.ActivationFunctionType.Copy,
                            bias=float(b3val), scale=1.0,
                        )
                        # deltas out: both col-halves; host selects by parity
                        nc.sync.dma_start(
                            out=out_p[t, p * G : (p + 1) * G, :],
                            in_=d_new[0:G, :],
                        )
                        h2prev[ln] = h2
    return nc


LAST_RESULT = None


def kernel(**inputs):
    return _run(inputs, NPASS, T)


def _prepare(inputs, npass, nsteps):
    features = np.asarray(inputs["features"], dtype=np.float32)
    W1 = np.asarray(inputs["W1"], dtype=np.float32)
    b1 = np.asarray(inputs["b1"], dtype=np.float32)
    W2 = np.asarray(inputs["W2"], dtype=np.float32)
    b2 = np.asarray(inputs["b2"], dtype=np.float32)
    W3 = np.asarray(inputs["W3"], dtype=np.float32)
    b3 = np.asarray(inputs["b3"], dtype=np.float32)

    nc = _build_graph(npass, nsteps, float(b3[0]))
    nc.finalize()

    # host-side packing
    bf = ml_dtypes.bfloat16
    wpack = np.zeros((128, WCOLS), np.float32)
    M = W3 @ W1[3:4]  # (64, 64) rank-1: M[i, j] = W3[i] * W1[3, j]
    for half in (0, 64):
        wpack[half : half + 3, W1A_OFF : W1A_OFF + HID] = W1[0:3]
        wpack[half : half + HID, M_OFF : M_OFF + HID] = M
        wpack[half : half + HID, W2_OFF : W2_OFF + HID] = W2
        for c in range(G):
            wpack[half : half + HID, W3_OFF + 32 * c + c] = W3[:, 0]
    wpack = wpack.astype(bf)

    b1p = b1 + W1[3] * b3[0]
    biasp = np.zeros((128, 4), np.float32)
    for half in (0, 64):
        biasp[half : half + HID, 0] = b1p
        biasp[half : half + HID, 1] = b2
        biasp[half : half + HID, 3] = b1
    biasp[:, 2] = b3[0]

    in_maps = []
    for k in range(NCORES):
        sh = features[k * NC : (k + 1) * NC]          # (NC, T, 3)
        feats = np.ascontiguousarray(sh.transpose(1, 2, 0)).astype(bf)  # (T,3,NC)
        in_maps.append({"feats": feats, "wpack": wpack, "biasp": biasp})

    return nc, in_maps


def _run(inputs, npass, nsteps, trace=False):
    global LAST_RESULT
    from concourse.bass_utils import run_bass_kernel_spmd

    nc, in_maps = _prepare(inputs, npass, nsteps)
    res = run_bass_kernel_spmd(
        nc, in_maps, core_ids=list(range(NCORES)), trace=trace
    )
    LAST_RESULT = res
    outs = res.results

    full = np.empty((N_TOT, T), np.float32)
    rows = np.arange(NPASS * G)
    par = rows % 2
    for k in range(NCORES):
        o = np.asarray(outs[k]["out"]).astype(np.float32)  # (T, 64, 2*CH)
        o = o.reshape(T, NPASS * G, 2, CH)[:, rows, par, :]  # (T, 64, CH)
        full[k * NC : (k + 1) * NC, :] = o.reshape(T, NC).T
    return full


if __name__ == "__main__":
    import reference

    inputs = reference.setup_inputs()
    out = kernel(**{k: np.asarray(v) for k, v in inputs.items()})
    print("kernel out", out.shape, out.dtype)



# revision 5
# speedup vs baseline: 1.1995x; 1.1995x over previous
"""Trainium2 Bass kernel for BaselineFeedforwardNetwork (dense_mlp).

Computation (per path n, step t):
    x_t   = [f_t (3), delta_{t-1} (1)]
    h     = relu(x_t @ W1 + b1)        # 4  -> 64
    h2    = relu(h @ W2 + b2)          # 64 -> 64
    delta = h2 @ W3 + b3               # 64 -> 1
Output: deltas (N, T).

Strategy (8 NeuronCores, pure data parallel over N):
  * hidden dim on SBUF partitions, paths on the free axis
  * per core: 32768 paths, processed as 4 pass-pairs of 2 lanes x 4096
    paths (8 chunks x 512); the delta feedback is folded into layer 1
    as a rank-1 matrix M = W3 @ W1[3,:] consuming h2_{t-1} (avoids a
    per-step PSUM->SBUF delta copy on the critical path)
  * all matmul operands bf16
  * per lane one in-place PSUM tile (4 banks): pre1 -> pre2 -> deltas
  * matmul emission is stage-major and interleaved across lanes /
    chunk parities so consecutive PE instructions sit on disjoint
    32x32 quadrant sets (lane 0 diagonal, lane 1 anti-diagonal) and
    can overlap in the array
  * mm3 packs all 8 chunk deltas into one [8, 512] PSUM region (one
    accumulation group, select-column lhsT), lane 0 at col-group 0,
    lane 1 at col-group 32; act3 is a single [8,512] copy per lane
"""

import sys

for _p in ("/opt/trn_rl_repo",):
    if _p not in sys.path:
        sys.path.insert(0, _p)

import os
import numpy as np
import ml_dtypes

NCORES = 8
N_TOT, T, FDIM = 262144, 60, 3
NC = N_TOT // NCORES          # 32768 paths per core
HID = 64
CH = 512                      # matmul free dim (one PSUM bank of fp32)
G = 8                         # chunks per pass-step
GP = G * CH                   # 4096 paths per pass
NPASS = NC // GP              # 8
NLANES = 2                    # passes in lockstep

# wpack column layout (all bf16, 128 partitions; every block duplicated on
# both partition halves so any chunk parity / lane can read it)
W1A_OFF = 0                                  # rows {0:3, 64:67} = W1[0:3]
M_OFF = 64                                   # rows 0:64 and 64:128 = W3 @ W1[3,:] (rank-1 fold)
W2_OFF = M_OFF + 64                          # rows 0:64 and 64:128 = W2
W3_OFF = W2_OFF + 64                         # [*, W3_OFF+32c+c] = W3 iff select col == c (dup halves)
WCOLS = W3_OFF + 32 * G                      # 448

DBANK = 3                                    # PSUM bank (col block) holding packed deltas


def _build_graph(npass=NPASS, nsteps=T, b3val=0.0):
    import concourse.bacc as bacc
    from concourse import mybir
    from concourse.tile import TileContext

    BF = mybir.dt.bfloat16
    F32 = mybir.dt.float32

    import time as _time

    nc = bacc.Bacc(trn_type="TRN2", name=f"k{int(_time.time())}")

    feats_p = nc.declare_dram_parameter("feats", [T, FDIM, NC], BF, isOutput=False)
    wpack_p = nc.declare_dram_parameter("wpack", [128, WCOLS], BF, isOutput=False)
    bias_p = nc.declare_dram_parameter("biasp", [128, 4], F32, isOutput=False)
    out_p = nc.declare_dram_parameter("out", [T, NPASS * G, 2 * CH], BF, isOutput=True)

    with TileContext(nc) as tc:
        with (
            tc.tile_pool(name="consts", bufs=1) as cpool,
            tc.tile_pool(name="sbuf", bufs=2) as spool,
            tc.tile_pool(name="xqp", bufs=6) as xpool,
            tc.tile_pool(name="psum", bufs=1, space="PSUM") as ppool,
        ):
            wp = cpool.tile([128, WCOLS], BF, tag="wpack")
            bp = cpool.tile([128, 4], F32, tag="biasp")
            nc.sync.dma_start(out=wp[:, :], in_=wpack_p[:, :])
            nc.sync.dma_start(out=bp[:, :], in_=bias_p[:, :])

            # Warm-up: loads the ACT table + lets ACT/DVE observe const DMAs
            warm = cpool.tile([128, 4], F32, tag="warm")
            nc.scalar.activation(
                warm[:, 0:1], bp[:, 0:1],
                mybir.ActivationFunctionType.Relu, bias=0.0, scale=1.0,
            )
            nc.vector.tensor_scalar(
                warm[:, 1:2], bp[:, 1:2], 0.0, None, mybir.AluOpType.add,
            )

            def dma_x(ln, p, t0):
                xt = xpool.tile([67, GP], BF, tag=f"xq{ln}")
                nc.sync.dma_start(
                    out=xt[0:FDIM, :], in_=feats_p[t0, :, p * GP : (p + 1) * GP]
                )
                nc.sync.dma_start(
                    out=xt[64 : 64 + FDIM, :],
                    in_=feats_p[t0, :, p * GP : (p + 1) * GP],
                )
                return xt

            XPRE = 5  # steps of feature prefetch
            for ppair in range(npass // NLANES):
                lanes = [ppair * NLANES + ln for ln in range(NLANES)]
                xq = [[dma_x(ln, p, t0) for t0 in range(min(XPRE, nsteps))]
                      for ln, p in enumerate(lanes)]
                h2prev = [None] * NLANES
                def mk_tile(pool, shape, dt, tag):
                    tmp = pool.tile(shape, dt, tag=tag)
                    return tmp

                for t in range(nsteps):
                    P = [None] * NLANES
                    h = [None] * NLANES
                    h2 = [None] * NLANES
                    d_new = [None] * NLANES
                    x = [None] * NLANES
                    for ln, p in enumerate(lanes):
                        if t + XPRE < nsteps:
                            xq[ln].append(dma_x(ln, p, t + XPRE))
                        x[ln] = xq[ln][t]
                        P[ln] = mk_tile(ppool, [128, (G // 2) * CH], F32, f"pp{ln}")
                        h[ln] = mk_tile(spool, [128, (G // 2) * CH], BF, f"h{ln}")
                        h2[ln] = mk_tile(spool, [128, (G // 2) * CH], BF, f"h2{ln}")
                        d_new[ln] = mk_tile(spool, [8, 2 * CH], BF, f"d{ln}")

                    # lane-dependent partition parities: lane 0 uses the
                    # diagonal PE quadrants, lane 1 the anti-diagonal, so
                    # interleaved emission covers disjoint subarrays
                    def pH(c, ln):   # pre1 / h partitions
                        return 64 * ((c % 2) ^ ln)

                    def pH2(c):  # pre2 / h2 partitions (and x row copy)
                        return 64 * (c % 2)

                    # ---- layer 1: pre1 = W1a^T f (+ M^T h2prev) ----
                    # emission order rotates the 4 quadrant sets:
                    # l0c_even(ll), l0c_odd(hh), l1c_even(lh), l1c_odd(hl)
                    for pair in range(G // 2):
                        for ln in range(NLANES):
                            for par in range(2):
                                c = 2 * pair + par
                                blk = pair * CH
                                o = P[ln][pH(c, ln) : pH(c, ln) + HID, blk : blk + CH]
                                tp = (pH2(c), pH(c, ln))
                                nc.tensor.matmul(
                                    o,
                                    wp[pH2(c) : pH2(c) + FDIM, W1A_OFF : W1A_OFF + HID],
                                    x[ln][pH2(c) : pH2(c) + FDIM, c * CH : (c + 1) * CH],
                                    start=True,
                                    stop=(t == 0),
                                    tile_position=tp,
                                )
                                if t > 0:
                                    nc.tensor.matmul(
                                        o,
                                        wp[pH2(c) : pH2(c) + HID, M_OFF : M_OFF + HID],
                                        h2prev[ln][pH2(c) : pH2(c) + HID, blk : blk + CH],
                                        start=False,
                                        stop=True,
                                        tile_position=tp,
                                    )
                    # ---- act1: h = relu(pre1 + b1') on ACT ----
                    # t=0 uses plain b1 (no delta yet); t>0 uses
                    # b1' = b1 + W1[3,:]*b3 (completes the rank-1 fold)
                    for ln in range(NLANES):
                        nc.scalar.activation(
                            h[ln][:, :], P[ln][:, :],
                            mybir.ActivationFunctionType.Relu,
                            bias=bp[:, 3:4] if t == 0 else bp[:, 0:1],
                            scale=1.0,
                        )
                    # ---- layer 2 (interleaved like layer 1) ----
                    for pair in range(G // 2):
                        for ln in range(NLANES):
                            for par in range(2):
                                c = 2 * pair + par
                                blk = pair * CH
                                nc.tensor.matmul(
                                    P[ln][pH2(c) : pH2(c) + HID, blk : blk + CH],
                                    wp[pH(c, ln) : pH(c, ln) + HID, W2_OFF : W2_OFF + HID],
                                    h[ln][pH(c, ln) : pH(c, ln) + HID, blk : blk + CH],
                                    start=True,
                                    stop=True,
                                    tile_position=(pH(c, ln), pH2(c)),
                                )
                    # ---- act2: h2 = relu(pre2 + b2) on DVE ----
                    for ln in range(NLANES):
                        nc.vector.tensor_scalar(
                            h2[ln][:, :], P[ln][:, :],
                            bp[:, 1:2], 0.0,
                            mybir.AluOpType.add, mybir.AluOpType.max,
                        )
                    # ---- layer 3 select: chunk deltas -> PSUM rows ----
                    # lane0 rows 0:8, lane1 rows 32:40; even chunks into
                    # cols 0:CH of bank 2, odd into bank 3; uniform
                    # positions per accumulation group
                    for ln in range(NLANES):
                        dr = 32 * ln
                        for par in range(2):
                            cs = [c for c in range(G) if c % 2 == par]
                            dblk = (2 + par) * CH
                            for i, c in enumerate(cs):
                                nc.tensor.matmul(
                                    P[ln][dr : dr + 32, dblk : dblk + CH],
                                    wp[pH2(c) : pH2(c) + HID, W3_OFF + 32 * c : W3_OFF + 32 * (c + 1)],
                                    h2[ln][pH2(c) : pH2(c) + HID, (c // 2) * CH : (c // 2) * CH + CH],
                                    start=(i == 0),
                                    stop=(i == len(cs) - 1),
                                    tile_position=(pH2(c), dr),
                                )
                    # ---- act3: delta = deltapre + b3 (on ACT) ----
                    for ln, p in enumerate(lanes):
                        dr = 32 * ln
                        nc.scalar.activation(
                            d_new[ln][0:G, :], P[ln][dr : dr + G, 2 * CH : 4 * CH],
                            mybir.ActivationFunctionType.Copy,
                            bias=float(b3val), scale=1.0,
                        )
                        nc.sync.dma_start(
                            out=out_p[t, p * G : (p + 1) * G, :],
                            in_=d_new[ln][0:G, :],
                        )
                        h2prev[ln] = h2[ln]
    return nc


LAST_RESULT = None


def kernel(**inputs):
    return _run(inputs, NPASS, T)


def _prepare(inputs, npass, nsteps):
    features = np.asarray(inputs["features"], dtype=np.float32)
    W1 = np.asarray(inputs["W1"], dtype=np.float32)
    b1 = np.asarray(inputs["b1"], dtype=np.float32)
    W2 = np.asarray(inputs["W2"], dtype=np.float32)
    b2 = np.asarray(inputs["b2"], dtype=np.float32)
    W3 = np.asarray(inputs["W3"], dtype=np.float32)
    b3 = np.asarray(inputs["b3"], dtype=np.float32)

    nc = _build_graph(npass, nsteps, float(b3[0]))
    nc.finalize()

    # host-side packing
    bf = ml_dtypes.bfloat16
    wpack = np.zeros((128, WCOLS), np.float32)
    M = W3 @ W1[3:4]  # (64, 64) rank-1: M[i, j] = W3[i] * W1[3, j]
    for half in (0, 64):
        wpack[half : half + 3, W1A_OFF : W1A_OFF + HID] = W1[0:3]
        wpack[half : half + HID, M_OFF : M_OFF + HID] = M
        wpack[half : half + HID, W2_OFF : W2_OFF + HID] = W2
        for c in range(G):
            wpack[half : half + HID, W3_OFF + 32 * c + c] = W3[:, 0]
    wpack = wpack.astype(bf)

    b1p = b1 + W1[3] * b3[0]
    biasp = np.zeros((128, 4), np.float32)
    for half in (0, 64):
        biasp[half : half + HID, 0] = b1p
        biasp[half : half + HID, 1] = b2
        biasp[half : half + HID, 3] = b1
    biasp[:, 2] = b3[0]

    in_maps = []
    for k in range(NCORES):
        sh = features[k * NC : (k + 1) * NC]          # (NC, T, 3)
        feats = np.ascontiguousarray(sh.transpose(1, 2, 0)).astype(bf)  # (T,3,NC)
        in_maps.append({"feats": feats, "wpack": wpack, "biasp": biasp})

    return nc, in_maps


def _run(inputs, npass, nsteps, trace=False):
    global LAST_RESULT
    from concourse.bass_utils import run_bass_kernel_spmd

    nc, in_maps = _prepare(inputs, npass, nsteps)
    res = run_bass_kernel_spmd(
        nc, in_maps, core_ids=list(range(NCORES)), trace=trace
    )
    LAST_RESULT = res
    outs = res.results

    full = np.empty((N_TOT, T), np.float32)
    rows = np.arange(NPASS * G)
    par = rows % 2
    for k in range(NCORES):
        o = np.asarray(outs[k]["out"]).astype(np.float32)  # (T, 64, 2*CH)
        o = o.reshape(T, NPASS * G, 2, CH)[:, rows, par, :]  # (T, 64, CH)
        full[k * NC : (k + 1) * NC, :] = o.reshape(T, NC).T
    return full


if __name__ == "__main__":
    import reference

    inputs = reference.setup_inputs()
    out = kernel(**{k: np.asarray(v) for k, v in inputs.items()})
    print("kernel out", out.shape, out.dtype)


# revision 6
# speedup vs baseline: 2.0475x; 1.7071x over previous
"""Trainium2 Bass kernel for BaselineFeedforwardNetwork (dense_mlp).

Computation (per path n, step t):
    x_t   = [f_t (3), delta_{t-1} (1)]
    h     = relu(x_t @ W1 + b1)        # 4  -> 64
    h2    = relu(h @ W2 + b2)          # 64 -> 64
    delta = h2 @ W3 + b3               # 64 -> 1
Output: deltas (N, T).

Strategy (8 NeuronCores, pure data parallel over N):
  * hidden dim on SBUF partitions, paths on the free axis
  * per core: 32768 paths, processed as 4 pass-pairs of 2 lanes x 4096
    paths (8 chunks x 512); the delta feedback is folded into layer 1
    as a rank-1 matrix M = W3 @ W1[3,:] consuming h2_{t-1} (avoids a
    per-step PSUM->SBUF delta copy on the critical path)
  * all matmul operands bf16
  * per lane one in-place PSUM tile (4 banks): pre1 -> pre2 -> deltas
  * matmul emission is stage-major and interleaved across lanes /
    chunk parities so consecutive PE instructions sit on disjoint
    32x32 quadrant sets (lane 0 diagonal, lane 1 anti-diagonal) and
    can overlap in the array
  * mm3 packs all 8 chunk deltas into one [8, 512] PSUM region (one
    accumulation group, select-column lhsT), lane 0 at col-group 0,
    lane 1 at col-group 32; act3 is a single [8,512] copy per lane
"""

import sys

for _p in ("/opt/trn_rl_repo",):
    if _p not in sys.path:
        sys.path.insert(0, _p)

import os
import numpy as np
import ml_dtypes

NCORES = 8
N_TOT, T, FDIM = 262144, 60, 3
NC = N_TOT // NCORES          # 32768 paths per core
HID = 64
CH = 512                      # matmul free dim (one PSUM bank of fp32)
G = 8                         # chunks per pass-step
GP = G * CH                   # 4096 paths per pass
NPASS = NC // GP              # 8
NLANES = 2                    # passes in lockstep

# wpack column layout (all bf16, 128 partitions; every block duplicated on
# both partition halves so any chunk parity / lane can read it)
W1A_OFF = 0                                  # rows {0:3, 64:67} = W1[0:3]
M_OFF = 64                                   # rows 0:64 and 64:128 = W3 @ W1[3,:] (rank-1 fold)
W2_OFF = M_OFF + 64                          # rows 0:64 and 64:128 = W2
W3_OFF = W2_OFF + 64                         # [*, W3_OFF+32c+c] = W3 iff select col == c (dup halves)
WCOLS = W3_OFF + 32 * G                      # 448

DBANK = 3                                    # PSUM bank (col block) holding packed deltas


def _build_graph(npass=NPASS, nsteps=T, b3val=0.0):
    import concourse.bacc as bacc
    from concourse import mybir
    from concourse.tile import TileContext

    BF = mybir.dt.bfloat16
    F32 = mybir.dt.float32

    import time as _time

    nc = bacc.Bacc(trn_type="TRN2", name=f"k{int(_time.time())}")

    feats_p = nc.declare_dram_parameter("feats", [T, FDIM, NC], BF, isOutput=False)
    wpack_p = nc.declare_dram_parameter("wpack", [128, WCOLS], BF, isOutput=False)
    bias_p = nc.declare_dram_parameter("biasp", [128, 4], F32, isOutput=False)
    out_p = nc.declare_dram_parameter("out", [T, 2, NPASS * G // 2, CH], BF, isOutput=True)

    with TileContext(nc) as tc:
        with (
            tc.tile_pool(name="consts", bufs=1) as cpool,
            tc.tile_pool(name="sbuf", bufs=2) as spool,
            tc.tile_pool(name="xqp", bufs=6) as xpool,
            tc.tile_pool(name="psum", bufs=1, space="PSUM") as ppool,
        ):
            wp = cpool.tile([128, WCOLS], BF, tag="wpack")
            bp = cpool.tile([128, 4], F32, tag="biasp")
            nc.sync.dma_start(out=wp[:, :], in_=wpack_p[:, :])
            nc.sync.dma_start(out=bp[:, :], in_=bias_p[:, :])

            # Warm-up: loads the ACT table + lets ACT/DVE observe const DMAs
            warm = cpool.tile([128, 4], F32, tag="warm")
            nc.scalar.activation(
                warm[:, 0:1], bp[:, 0:1],
                mybir.ActivationFunctionType.Relu, bias=0.0, scale=1.0,
            )
            nc.vector.tensor_scalar(
                warm[:, 1:2], bp[:, 1:2], 0.0, None, mybir.AluOpType.add,
            )

            def dma_x(ln, p, t0):
                xt = xpool.tile([67, GP], BF, tag=f"xq{ln}")
                nc.sync.dma_start(
                    out=xt[0:FDIM, :], in_=feats_p[t0, :, p * GP : (p + 1) * GP]
                )
                nc.sync.dma_start(
                    out=xt[64 : 64 + FDIM, :],
                    in_=feats_p[t0, :, p * GP : (p + 1) * GP],
                )
                return xt

            XPRE = 5  # steps of feature prefetch
            for ppair in range(npass // NLANES):
                lanes = [ppair * NLANES + ln for ln in range(NLANES)]
                xq = [[dma_x(ln, p, t0) for t0 in range(min(XPRE, nsteps))]
                      for ln, p in enumerate(lanes)]
                h2prev = [None] * NLANES
                def mk_tile(pool, shape, dt, tag):
                    tmp = pool.tile(shape, dt, tag=tag)
                    return tmp

                for t in range(nsteps):
                    P = [None] * NLANES
                    h = [None] * NLANES
                    h2 = [None] * NLANES
                    d_new = [None] * NLANES
                    x = [None] * NLANES
                    for ln, p in enumerate(lanes):
                        if t + XPRE < nsteps:
                            xq[ln].append(dma_x(ln, p, t + XPRE))
                        x[ln] = xq[ln][t]
                        P[ln] = mk_tile(ppool, [128, (G // 2) * CH], F32, f"pp{ln}")
                        h[ln] = mk_tile(spool, [128, (G // 2) * CH], BF, f"h{ln}")
                        h2[ln] = mk_tile(spool, [128, (G // 2) * CH], BF, f"h2{ln}")
                        d_new[ln] = mk_tile(spool, [36, CH], BF, f"d{ln}")

                    # lane-dependent partition parities: lane 0 uses the
                    # diagonal PE quadrants, lane 1 the anti-diagonal, so
                    # interleaved emission covers disjoint subarrays
                    def pH(c, ln):   # pre1 / h partitions
                        return 64 * ((c % 2) ^ ln)

                    def pH2(c):  # pre2 / h2 partitions (and x row copy)
                        return 64 * (c % 2)

                    # ---- layer 1: pre1 = W1a^T f (+ M^T h2prev) ----
                    # two full rotations over the 4 disjoint quadrant sets
                    # (l0even, l0odd, l1even, l1odd) so consecutive PE
                    # instructions overlap; the per-chunk accumulation
                    # group (mm1a start -> mmM stop) is interleaved across
                    # banks, which is legal since zero regions differ
                    for pair in range(G // 2):
                        for ln in range(NLANES):
                            for par in range(2):
                                c = 2 * pair + par
                                blk = pair * CH
                                o = P[ln][pH(c, ln) : pH(c, ln) + HID, blk : blk + CH]
                                nc.tensor.matmul(
                                    o,
                                    wp[pH2(c) : pH2(c) + FDIM, W1A_OFF : W1A_OFF + HID],
                                    x[ln][pH2(c) : pH2(c) + FDIM, c * CH : (c + 1) * CH],
                                    start=True,
                                    stop=(t == 0),
                                    tile_position=(pH2(c), pH(c, ln)),
                                )
                    if t > 0:
                        for pair in range(G // 2):
                            for ln in range(NLANES):
                                for par in range(2):
                                    c = 2 * pair + par
                                    blk = pair * CH
                                    o = P[ln][pH(c, ln) : pH(c, ln) + HID, blk : blk + CH]
                                    nc.tensor.matmul(
                                        o,
                                        wp[pH2(c) : pH2(c) + HID, M_OFF : M_OFF + HID],
                                        h2prev[ln][pH2(c) : pH2(c) + HID, blk : blk + CH],
                                        start=False,
                                        stop=True,
                                        tile_position=(pH2(c), pH(c, ln)),
                                    )
                    # ---- act1: h = relu(pre1 + b1') on ACT ----
                    # t=0 uses plain b1 (no delta yet); t>0 uses
                    # b1' = b1 + W1[3,:]*b3 (completes the rank-1 fold)
                    for ln in range(NLANES):
                        nc.scalar.activation(
                            h[ln][:, :], P[ln][:, :],
                            mybir.ActivationFunctionType.Relu,
                            bias=bp[:, 3:4] if t == 0 else bp[:, 0:1],
                            scale=1.0,
                        )
                    # ---- layer 2 (interleaved like layer 1) ----
                    for pair in range(G // 2):
                        for ln in range(NLANES):
                            for par in range(2):
                                c = 2 * pair + par
                                blk = pair * CH
                                nc.tensor.matmul(
                                    P[ln][pH2(c) : pH2(c) + HID, blk : blk + CH],
                                    wp[pH(c, ln) : pH(c, ln) + HID, W2_OFF : W2_OFF + HID],
                                    h[ln][pH(c, ln) : pH(c, ln) + HID, blk : blk + CH],
                                    start=True,
                                    stop=True,
                                    tile_position=(pH(c, ln), pH2(c)),
                                )
                    # ---- act2: h2 = relu(pre2 + b2) on DVE ----
                    for ln in range(NLANES):
                        nc.vector.tensor_scalar(
                            h2[ln][:, :], P[ln][:, :],
                            bp[:, 1:2], 0.0,
                            mybir.AluOpType.add, mybir.AluOpType.max,
                        )
                    # ---- layer 3 select: chunk deltas -> PSUM rows ----
                    # 4 accumulation groups (lane x parity), each in its
                    # own 32-row col-group of bank 3 so the rotation
                    # l0p0,l0p1,l1p0,l1p1 covers 4 disjoint quadrant sets;
                    # chunk c lands on row dr + c//2 (select col c//2)
                    dblk = 3 * CH
                    for i in range(G // 2):
                        for ln in range(NLANES):
                            for par in range(2):
                                c = 2 * i + par
                                dr = 32 * (2 * ln + par)
                                nc.tensor.matmul(
                                    P[ln][dr : dr + 32, dblk : dblk + CH],
                                    wp[pH2(c) : pH2(c) + HID, W3_OFF + 32 * c : W3_OFF + 32 * (c + 1)],
                                    h2[ln][pH2(c) : pH2(c) + HID, (c // 2) * CH : (c // 2) * CH + CH],
                                    start=(i == 0),
                                    stop=(i == G // 2 - 1),
                                    tile_position=(pH2(c), dr),
                                )
                    # ---- act3: delta = deltapre + b3 (on ACT) ----
                    # lane0 rows 0:36 (par0 at 0:4, par1 at 32:36), lane1
                    # rows 64:100 of its own P tile; one [36,512] copy each
                    for ln, p in enumerate(lanes):
                        dr0 = 64 * ln
                        nc.scalar.activation(
                            d_new[ln][0:36, :], P[ln][dr0 : dr0 + 36, dblk : dblk + CH],
                            mybir.ActivationFunctionType.Copy,
                            bias=float(b3val), scale=1.0,
                        )
                        for par in range(2):
                            nc.sync.dma_start(
                                out=out_p[t, par, p * (G // 2) : (p + 1) * (G // 2), :],
                                in_=d_new[ln][32 * par : 32 * par + G // 2, :],
                            )
                        h2prev[ln] = h2[ln]
    return nc


LAST_RESULT = None


def kernel(**inputs):
    return _run(inputs, NPASS, T)


def _prepare(inputs, npass, nsteps):
    features = np.asarray(inputs["features"], dtype=np.float32)
    W1 = np.asarray(inputs["W1"], dtype=np.float32)
    b1 = np.asarray(inputs["b1"], dtype=np.float32)
    W2 = np.asarray(inputs["W2"], dtype=np.float32)
    b2 = np.asarray(inputs["b2"], dtype=np.float32)
    W3 = np.asarray(inputs["W3"], dtype=np.float32)
    b3 = np.asarray(inputs["b3"], dtype=np.float32)

    nc = _build_graph(npass, nsteps, float(b3[0]))
    nc.finalize()

    # host-side packing
    bf = ml_dtypes.bfloat16
    wpack = np.zeros((128, WCOLS), np.float32)
    M = W3 @ W1[3:4]  # (64, 64) rank-1: M[i, j] = W3[i] * W1[3, j]
    for half in (0, 64):
        wpack[half : half + 3, W1A_OFF : W1A_OFF + HID] = W1[0:3]
        wpack[half : half + HID, M_OFF : M_OFF + HID] = M
        wpack[half : half + HID, W2_OFF : W2_OFF + HID] = W2
        for c in range(G):
            wpack[half : half + HID, W3_OFF + 32 * c + c // 2] = W3[:, 0]
    wpack = wpack.astype(bf)

    b1p = b1 + W1[3] * b3[0]
    biasp = np.zeros((128, 4), np.float32)
    for half in (0, 64):
        biasp[half : half + HID, 0] = b1p
        biasp[half : half + HID, 1] = b2
        biasp[half : half + HID, 3] = b1
    biasp[:, 2] = b3[0]

    in_maps = []
    for k in range(NCORES):
        sh = features[k * NC : (k + 1) * NC]          # (NC, T, 3)
        feats = np.ascontiguousarray(sh.transpose(1, 2, 0)).astype(bf)  # (T,3,NC)
        in_maps.append({"feats": feats, "wpack": wpack, "biasp": biasp})

    return nc, in_maps


def _run(inputs, npass, nsteps, trace=False):
    global LAST_RESULT
    from concourse.bass_utils import run_bass_kernel_spmd

    nc, in_maps = _prepare(inputs, npass, nsteps)
    res = run_bass_kernel_spmd(
        nc, in_maps, core_ids=list(range(NCORES)), trace=trace
    )
    LAST_RESULT = res
    outs = res.results

    full = np.empty((N_TOT, T), np.float32)
    for k in range(NCORES):
        o = np.asarray(outs[k]["out"]).astype(np.float32)  # (T, 2, 32, CH)
        # o[t, par, p*4 + j, :] holds chunk c=2j+par of pass p
        o = o.reshape(T, 2, NPASS, G // 2, CH).transpose(0, 2, 3, 1, 4)
        full[k * NC : (k + 1) * NC, :] = o.reshape(T, NC).T
    return full


if __name__ == "__main__":
    import reference

    inputs = reference.setup_inputs()
    out = kernel(**{k: np.asarray(v) for k, v in inputs.items()})
    print("kernel out", out.shape, out.dtype)


# revision 7
# speedup vs baseline: 2.1526x; 1.0513x over previous
"""Trainium2 Bass kernel for BaselineFeedforwardNetwork (dense_mlp).

Computation (per path n, step t):
    x_t   = [f_t (3), delta_{t-1} (1)]
    h     = relu(x_t @ W1 + b1)        # 4  -> 64
    h2    = relu(h @ W2 + b2)          # 64 -> 64
    delta = h2 @ W3 + b3               # 64 -> 1
Output: deltas (N, T).

Strategy (8 NeuronCores, pure data parallel over N):
  * hidden dim on SBUF partitions, paths on the free axis
  * per core: 32768 paths, processed as 4 pass-pairs of 2 lanes x 4096
    paths (8 chunks x 512); the delta feedback is folded into layer 1
    as a rank-1 matrix M = W3 @ W1[3,:] consuming h2_{t-1} (avoids a
    per-step PSUM->SBUF delta copy on the critical path)
  * all matmul operands bf16
  * per lane one in-place PSUM tile (4 banks): pre1 -> pre2 -> deltas
  * matmul emission is stage-major and interleaved across lanes /
    chunk parities so consecutive PE instructions sit on disjoint
    32x32 quadrant sets (lane 0 diagonal, lane 1 anti-diagonal) and
    can overlap in the array
  * mm3 packs all 8 chunk deltas into one [8, 512] PSUM region (one
    accumulation group, select-column lhsT), lane 0 at col-group 0,
    lane 1 at col-group 32; act3 is a single [8,512] copy per lane
"""

import sys

for _p in ("/opt/trn_rl_repo",):
    if _p not in sys.path:
        sys.path.insert(0, _p)

import os
import numpy as np
import ml_dtypes

NCORES = 8
N_TOT, T, FDIM = 262144, 60, 3
NC = N_TOT // NCORES          # 32768 paths per core
HID = 64
CH = 512                      # matmul free dim (one PSUM bank of fp32)
G = 8                         # chunks per pass-step
GP = G * CH                   # 4096 paths per pass
NPASS = NC // GP              # 8
NLANES = 2                    # passes in lockstep

# wpack column layout (all bf16, 128 partitions; every block duplicated on
# both partition halves so any chunk parity / lane can read it)
W1A_OFF = 0                                  # rows {0:3, 64:67} = W1[0:3]
M_OFF = 64                                   # rows 0:64 and 64:128 = W3 @ W1[3,:] (rank-1 fold)
W2_OFF = M_OFF + 64                          # rows 0:64 and 64:128 = W2
W3_OFF = W2_OFF + 64                         # [*, W3_OFF+32c+c] = W3 iff select col == c (dup halves)
WCOLS = W3_OFF + 32 * (G // 2)               # 320

DBANK = 3                                    # PSUM bank (col block) holding packed deltas


def _build_graph(npass=NPASS, nsteps=T, b3val=0.0):
    import concourse.bacc as bacc
    from concourse import mybir
    from concourse.tile import TileContext

    BF = mybir.dt.bfloat16
    F32 = mybir.dt.float32

    import time as _time

    nc = bacc.Bacc(trn_type="TRN2", name=f"k{int(_time.time())}")

    feats_p = nc.declare_dram_parameter("feats", [T, FDIM, NC], BF, isOutput=False)
    wpack_p = nc.declare_dram_parameter("wpack", [128, WCOLS], BF, isOutput=False)
    bias_p = nc.declare_dram_parameter("biasp", [128, 4], F32, isOutput=False)
    out_p = nc.declare_dram_parameter("out", [T, 2, NPASS * G // 2, CH], BF, isOutput=True)

    with TileContext(nc) as tc:
        with (
            tc.tile_pool(name="consts", bufs=1) as cpool,
            tc.tile_pool(name="sbuf", bufs=2) as spool,
            tc.tile_pool(name="xqp", bufs=6) as xpool,
            tc.tile_pool(name="psum", bufs=1, space="PSUM") as ppool,
        ):
            wp = cpool.tile([128, WCOLS], BF, tag="wpack")
            bp = cpool.tile([128, 4], F32, tag="biasp")
            nc.sync.dma_start(out=wp[:, :], in_=wpack_p[:, :])
            nc.sync.dma_start(out=bp[:, :], in_=bias_p[:, :])

            # Warm-up: loads the ACT table + lets ACT/DVE observe const DMAs
            warm = cpool.tile([128, 4], F32, tag="warm")
            nc.scalar.activation(
                warm[:, 0:1], bp[:, 0:1],
                mybir.ActivationFunctionType.Relu, bias=0.0, scale=1.0,
            )
            nc.vector.tensor_scalar(
                warm[:, 1:2], bp[:, 1:2], 0.0, None, mybir.AluOpType.add,
            )

            def dma_x(ln, p, t0):
                xt = xpool.tile([67, GP], BF, tag=f"xq{ln}")
                nc.sync.dma_start(
                    out=xt[0:FDIM, :], in_=feats_p[t0, :, p * GP : (p + 1) * GP]
                )
                nc.sync.dma_start(
                    out=xt[64 : 64 + FDIM, :],
                    in_=feats_p[t0, :, p * GP : (p + 1) * GP],
                )
                return xt

            XPRE = 5  # steps of feature prefetch
            for ppair in range(npass // NLANES):
                lanes = [ppair * NLANES + ln for ln in range(NLANES)]
                xq = [[dma_x(ln, p, t0) for t0 in range(min(XPRE, nsteps))]
                      for ln, p in enumerate(lanes)]
                h2prev = [None] * NLANES
                def mk_tile(pool, shape, dt, tag):
                    tmp = pool.tile(shape, dt, tag=tag)
                    return tmp

                for t in range(nsteps):
                    P = [None] * NLANES
                    h = [None] * NLANES
                    h2 = [None] * NLANES
                    d_new = [None] * NLANES
                    x = [None] * NLANES
                    for ln, p in enumerate(lanes):
                        if t + XPRE < nsteps:
                            xq[ln].append(dma_x(ln, p, t + XPRE))
                        x[ln] = xq[ln][t]
                        P[ln] = mk_tile(ppool, [128, (G // 2) * CH], F32, f"pp{ln}")
                        h[ln] = mk_tile(spool, [128, (G // 2) * CH], BF, f"h{ln}")
                        h2[ln] = mk_tile(spool, [128, (G // 2) * CH], BF, f"h2{ln}")
                        d_new[ln] = mk_tile(spool, [20, CH], BF, f"d{ln}")

                    # lane-dependent partition parities: lane 0 uses the
                    # diagonal PE quadrants, lane 1 the anti-diagonal, so
                    # interleaved emission covers disjoint subarrays
                    def pH(c, ln):   # pre1 / h partitions
                        return 64 * ((c % 2) ^ ln)

                    def pH2(c):  # pre2 / h2 partitions (and x row copy)
                        return 64 * (c % 2)

                    # ---- layer 1: pre1 = W1a^T f (+ M^T h2prev) ----
                    # two full rotations over the 4 disjoint quadrant sets
                    # (l0even, l0odd, l1even, l1odd) so consecutive PE
                    # instructions overlap; the per-chunk accumulation
                    # group (mm1a start -> mmM stop) is interleaved across
                    # banks, which is legal since zero regions differ
                    for pair in range(G // 2):
                        for ln in range(NLANES):
                            for par in range(2):
                                c = 2 * pair + par
                                blk = pair * CH
                                o = P[ln][pH(c, ln) : pH(c, ln) + HID, blk : blk + CH]
                                nc.tensor.matmul(
                                    o,
                                    wp[pH2(c) : pH2(c) + FDIM, W1A_OFF : W1A_OFF + HID],
                                    x[ln][pH2(c) : pH2(c) + FDIM, c * CH : (c + 1) * CH],
                                    start=True,
                                    stop=(t == 0),
                                    tile_position=(pH2(c), pH(c, ln)),
                                )
                    if t > 0:
                        for pair in range(G // 2):
                            for ln in range(NLANES):
                                for par in range(2):
                                    c = 2 * pair + par
                                    blk = pair * CH
                                    o = P[ln][pH(c, ln) : pH(c, ln) + HID, blk : blk + CH]
                                    nc.tensor.matmul(
                                        o,
                                        wp[pH2(c) : pH2(c) + HID, M_OFF : M_OFF + HID],
                                        h2prev[ln][pH2(c) : pH2(c) + HID, blk : blk + CH],
                                        start=False,
                                        stop=True,
                                        tile_position=(pH2(c), pH(c, ln)),
                                    )
                    # ---- act1: h = relu(pre1 + b1') ----
                    # lane 0 on ACT, lane 1 on DVE so both run in parallel.
                    # t=0 uses plain b1 (no delta yet); t>0 uses
                    # b1' = b1 + W1[3,:]*b3 (completes the rank-1 fold)
                    bcol = (3, 4) if t == 0 else (0, 1)
                    nc.scalar.activation(
                        h[0][:, :], P[0][:, :],
                        mybir.ActivationFunctionType.Relu,
                        bias=bp[:, bcol[0] : bcol[1]], scale=1.0,
                    )
                    nc.vector.tensor_scalar(
                        h[1][:, :], P[1][:, :],
                        bp[:, bcol[0] : bcol[1]], 0.0,
                        mybir.AluOpType.add, mybir.AluOpType.max,
                    )
                    # ---- layer 2 (interleaved like layer 1) ----
                    for pair in range(G // 2):
                        for ln in range(NLANES):
                            for par in range(2):
                                c = 2 * pair + par
                                blk = pair * CH
                                nc.tensor.matmul(
                                    P[ln][pH2(c) : pH2(c) + HID, blk : blk + CH],
                                    wp[pH(c, ln) : pH(c, ln) + HID, W2_OFF : W2_OFF + HID],
                                    h[ln][pH(c, ln) : pH(c, ln) + HID, blk : blk + CH],
                                    start=True,
                                    stop=True,
                                    tile_position=(pH(c, ln), pH2(c)),
                                )
                    # ---- act2: h2 = relu(pre2 + b2); lane 0 DVE, lane 1 ACT ----
                    nc.vector.tensor_scalar(
                        h2[0][:, :], P[0][:, :],
                        bp[:, 1:2], 0.0,
                        mybir.AluOpType.add, mybir.AluOpType.max,
                    )
                    nc.scalar.activation(
                        h2[1][:, :], P[1][:, :],
                        mybir.ActivationFunctionType.Relu,
                        bias=bp[:, 1:2], scale=1.0,
                    )
                    # ---- layer 3: K=128 pair-matmuls -> delta rows ----
                    # one MM per chunk pair computes both parities' deltas:
                    # lhsT [128,32] has W3 at (rows 0:64, col i) for chunk
                    # 2i and (rows 64:128, col 16+i) for chunk 2i+1. One
                    # accumulation group per lane (lane0 cg0, lane1 cg32),
                    # interleaved for 2-way overlap.
                    dblk = 3 * CH
                    for i in range(G // 2):
                        for ln in range(NLANES):
                            dr = 32 * ln
                            nc.tensor.matmul(
                                P[ln][dr : dr + 32, dblk : dblk + CH],
                                wp[0:128, W3_OFF + 32 * i : W3_OFF + 32 * (i + 1)],
                                h2[ln][0:128, i * CH : (i + 1) * CH],
                                start=(i == 0),
                                stop=(i == G // 2 - 1),
                                tile_position=(0, dr),
                            )
                    # ---- act3: delta = deltapre + b3; lane0 ACT, lane1 DVE ----
                    # even chunks at rows dr:dr+4, odd at dr+16:dr+20
                    for ln, p in enumerate(lanes):
                        dr = 32 * ln
                        if ln == 0:
                            nc.scalar.activation(
                                d_new[ln][0:20, :], P[ln][dr : dr + 20, dblk : dblk + CH],
                                mybir.ActivationFunctionType.Copy,
                                bias=float(b3val), scale=1.0,
                            )
                        else:
                            nc.vector.tensor_scalar(
                                d_new[ln][0:20, :], P[ln][dr : dr + 20, dblk : dblk + CH],
                                float(b3val), None, mybir.AluOpType.add,
                            )
                        for par in range(2):
                            nc.sync.dma_start(
                                out=out_p[t, par, p * (G // 2) : (p + 1) * (G // 2), :],
                                in_=d_new[ln][16 * par : 16 * par + G // 2, :],
                            )
                        h2prev[ln] = h2[ln]
    return nc


LAST_RESULT = None


def kernel(**inputs):
    return _run(inputs, NPASS, T)


def _prepare(inputs, npass, nsteps):
    features = np.asarray(inputs["features"], dtype=np.float32)
    W1 = np.asarray(inputs["W1"], dtype=np.float32)
    b1 = np.asarray(inputs["b1"], dtype=np.float32)
    W2 = np.asarray(inputs["W2"], dtype=np.float32)
    b2 = np.asarray(inputs["b2"], dtype=np.float32)
    W3 = np.asarray(inputs["W3"], dtype=np.float32)
    b3 = np.asarray(inputs["b3"], dtype=np.float32)

    nc = _build_graph(npass, nsteps, float(b3[0]))
    nc.finalize()

    # host-side packing
    bf = ml_dtypes.bfloat16
    wpack = np.zeros((128, WCOLS), np.float32)
    M = W3 @ W1[3:4]  # (64, 64) rank-1: M[i, j] = W3[i] * W1[3, j]
    for half in (0, 64):
        wpack[half : half + 3, W1A_OFF : W1A_OFF + HID] = W1[0:3]
        wpack[half : half + HID, M_OFF : M_OFF + HID] = M
        wpack[half : half + HID, W2_OFF : W2_OFF + HID] = W2
    for i in range(G // 2):
        wpack[0:HID, W3_OFF + 32 * i + i] = W3[:, 0]
        wpack[64 : 64 + HID, W3_OFF + 32 * i + 16 + i] = W3[:, 0]
    wpack = wpack.astype(bf)

    b1p = b1 + W1[3] * b3[0]
    biasp = np.zeros((128, 4), np.float32)
    for half in (0, 64):
        biasp[half : half + HID, 0] = b1p
        biasp[half : half + HID, 1] = b2
        biasp[half : half + HID, 3] = b1
    biasp[:, 2] = b3[0]

    in_maps = []
    for k in range(NCORES):
        sh = features[k * NC : (k + 1) * NC]          # (NC, T, 3)
        feats = np.ascontiguousarray(sh.transpose(1, 2, 0)).astype(bf)  # (T,3,NC)
        in_maps.append({"feats": feats, "wpack": wpack, "biasp": biasp})

    return nc, in_maps


def _run(inputs, npass, nsteps, trace=False):
    global LAST_RESULT
    from concourse.bass_utils import run_bass_kernel_spmd

    nc, in_maps = _prepare(inputs, npass, nsteps)
    res = run_bass_kernel_spmd(
        nc, in_maps, core_ids=list(range(NCORES)), trace=trace
    )
    LAST_RESULT = res
    outs = res.results

    full = np.empty((N_TOT, T), np.float32)
    for k in range(NCORES):
        o = np.asarray(outs[k]["out"]).astype(np.float32)  # (T, 2, 32, CH)
        # o[t, par, p*4 + j, :] holds chunk c=2j+par of pass p
        o = o.reshape(T, 2, NPASS, G // 2, CH).transpose(0, 2, 3, 1, 4)
        full[k * NC : (k + 1) * NC, :] = o.reshape(T, NC).T
    return full


if __name__ == "__main__":
    import reference

    inputs = reference.setup_inputs()
    out = kernel(**{k: np.asarray(v) for k, v in inputs.items()})
    print("kernel out", out.shape, out.dtype)


# revision 9
# speedup vs baseline: 2.3065x; 1.0715x over previous
"""Trainium2 Bass kernel for BaselineFeedforwardNetwork (dense_mlp).

Computation (per path n, step t):
    x_t   = [f_t (3), delta_{t-1} (1)]
    h     = relu(x_t @ W1 + b1)        # 4  -> 64
    h2    = relu(h @ W2 + b2)          # 64 -> 64
    delta = h2 @ W3 + b3               # 64 -> 1
Output: deltas (N, T).

Strategy (8 NeuronCores, pure data parallel over N):
  * hidden dim on SBUF partitions, paths on the free axis; two chunks
    of 512 paths stacked per 128 partitions
  * per core: 32768 paths = 16 passes of 2048; 4 passes ("vlanes") run
    concurrently, each owning 2 PSUM banks (in-place pre1->pre2->delta),
    so 4 independent per-step dependency chains overlap and keep all
    engines busy across the serial T recurrence
  * delta feedback folded into layer 1 as the rank-1 matrix
    M = W3 @ W1[3,:] consuming h2_{t-1} (no PSUM->SBUF copy on the
    feedback path)
  * matmul emission is stage-major, rotating vlane x chunk-parity so
    consecutive PE instructions hit disjoint 64x64 quadrant sets and
    overlap in the array (measured 65 ns/MM for 4-way rotation vs 436
    serial); per-chunk accumulation groups (mm1a start -> mmM stop)
    are interleaved across banks
  * mm3: one K=128 matmul per chunk pair computes both parities'
    deltas via a select-column lhsT; one accumulation group per vlane
    at col-group 32*v
  * activations alternate ACT/DVE per vlane to balance the engines
"""

import sys

for _p in ("/opt/trn_rl_repo",):
    if _p not in sys.path:
        sys.path.insert(0, _p)

import os
import numpy as np
import ml_dtypes

NCORES = 8
N_TOT, T, FDIM = 262144, 60, 3
NC = N_TOT // NCORES          # 32768 paths per core
HID = 64
CH = 512                      # matmul free dim (one PSUM bank of fp32)
G = 4                         # chunks per vlane-step
GP = G * CH                   # 2048 paths per pass
NPASS = NC // GP              # 16
NLANES = 4                    # passes in lockstep (2 PSUM banks each)

# wpack column layout (all bf16, 128 partitions; W1a/M/W2 duplicated on
# both partition halves so any chunk parity / vlane can read it)
W1A_OFF = 0                                  # rows {0:3, 64:67} = W1[0:3]
M_OFF = 64                                   # rows 0:64 / 64:128 = W3 @ W1[3,:] (rank-1 fold)
W2_OFF = M_OFF + 64                          # rows 0:64 / 64:128 = W2
W3_OFF = W2_OFF + 64                         # pair-select blocks [128,32]:
                                             #   block i: W3 at (rows 0:64, col i)
                                             #   and (rows 64:128, col 16+i)
WCOLS = W3_OFF + 32 * (G // 2)               # 256
DBANK = 1                                    # PSUM bank (col block) holding deltas


def _build_graph(npass=NPASS, nsteps=T, b3val=0.0):
    import concourse.bacc as bacc
    from concourse import mybir
    from concourse.tile import TileContext

    BF = mybir.dt.bfloat16
    F32 = mybir.dt.float32

    import time as _time

    nc = bacc.Bacc(trn_type="TRN2", name=f"k{int(_time.time())}")

    feats_p = nc.declare_dram_parameter("feats", [T, FDIM, NC], BF, isOutput=False)
    wpack_p = nc.declare_dram_parameter("wpack", [128, WCOLS], BF, isOutput=False)
    bias_p = nc.declare_dram_parameter("biasp", [128, 4], F32, isOutput=False)
    out_p = nc.declare_dram_parameter(
        "out", [T, 2, NPASS * G // 2, CH], BF, isOutput=True
    )

    NL = NLANES

    with TileContext(nc) as tc:
        with (
            tc.tile_pool(name="consts", bufs=1) as cpool,
            tc.tile_pool(name="sbuf", bufs=2) as spool,
            tc.tile_pool(name="xqp", bufs=6) as xpool,
            tc.tile_pool(name="psum", bufs=1, space="PSUM") as ppool,
        ):
            wp = cpool.tile([128, WCOLS], BF, tag="wpack")
            bp = cpool.tile([128, 4], F32, tag="biasp")
            nc.sync.dma_start(out=wp[:, :], in_=wpack_p[:, :])
            nc.sync.dma_start(out=bp[:, :], in_=bias_p[:, :])

            # Warm-up: loads the ACT table + lets ACT/DVE observe const DMAs
            warm = cpool.tile([128, 4], F32, tag="warm")
            nc.scalar.activation(
                warm[:, 0:1], bp[:, 0:1],
                mybir.ActivationFunctionType.Relu, bias=0.0, scale=1.0,
            )
            nc.vector.tensor_scalar(
                warm[:, 1:2], bp[:, 1:2], 0.0, None, mybir.AluOpType.add,
            )

            def dma_x(v, p, t0):
                xt = xpool.tile([67, GP], BF, tag=f"xq{v}")
                nc.sync.dma_start(
                    out=xt[0:FDIM, :], in_=feats_p[t0, :, p * GP : (p + 1) * GP]
                )
                nc.sync.dma_start(
                    out=xt[64 : 64 + FDIM, :],
                    in_=feats_p[t0, :, p * GP : (p + 1) * GP],
                )
                return xt

            def mk_tile(pool, shape, dt, tag):
                tmp = pool.tile(shape, dt, tag=tag)
                return tmp

            XPRE = 5  # steps of feature prefetch
            for rnd in range(npass // NL):
                lanes = [rnd * NL + v for v in range(NL)]
                xq = [[dma_x(v, p, t0) for t0 in range(min(XPRE, nsteps))]
                      for v, p in enumerate(lanes)]
                h2prev = [None] * NL
                for t in range(nsteps):
                    P = [None] * NL
                    h = [None] * NL
                    h2 = [None] * NL
                    d_new = [None] * NL
                    x = [None] * NL
                    for v, p in enumerate(lanes):
                        if t + XPRE < nsteps:
                            xq[v].append(dma_x(v, p, t + XPRE))
                        x[v] = xq[v][t]
                        P[v] = mk_tile(ppool, [128, (G // 2) * CH], F32, f"pp{v}")
                        h[v] = mk_tile(spool, [128, (G // 2) * CH], BF, f"h{v}")
                        h2[v] = mk_tile(spool, [128, (G // 2) * CH], BF, f"h2{v}")
                        d_new[v] = mk_tile(spool, [18, CH], BF, f"d{v}")

                    # chunk parity -> rhs partition half; vlane parity flips
                    # the output half so rotation covers all 4 quadrants
                    def pH(c, v):   # pre1 / h partitions
                        return 64 * ((c % 2) ^ (v % 2))

                    def pH2(c):  # pre2 / h2 partitions (and x row copy)
                        return 64 * (c % 2)

                    # ---- layer 1: pre1 = W1a^T f (+ M^T h2prev) ----
                    # two full rotations; per-chunk accumulation groups
                    # (mm1a start -> mmM stop) interleave across banks
                    for pair in range(G // 2):
                        for v in range(NL):
                            for par in range(2):
                                c = 2 * pair + par
                                blk = pair * CH
                                o = P[v][pH(c, v) : pH(c, v) + HID, blk : blk + CH]
                                nc.tensor.matmul(
                                    o,
                                    wp[pH2(c) : pH2(c) + FDIM, W1A_OFF : W1A_OFF + HID],
                                    x[v][pH2(c) : pH2(c) + FDIM, c * CH : (c + 1) * CH],
                                    start=True,
                                    stop=(t == 0),
                                    tile_position=(pH2(c), pH(c, v)),
                                )
                    if t > 0:
                        for pair in range(G // 2):
                            for v in range(NL):
                                for par in range(2):
                                    c = 2 * pair + par
                                    blk = pair * CH
                                    o = P[v][pH(c, v) : pH(c, v) + HID, blk : blk + CH]
                                    nc.tensor.matmul(
                                        o,
                                        wp[pH2(c) : pH2(c) + HID, M_OFF : M_OFF + HID],
                                        h2prev[v][pH2(c) : pH2(c) + HID, blk : blk + CH],
                                        start=False,
                                        stop=True,
                                        tile_position=(pH2(c), pH(c, v)),
                                    )
                    # ---- act1: h = relu(pre1 + b1') ----
                    # t=0 uses plain b1 (no delta yet); t>0 uses
                    # b1' = b1 + W1[3,:]*b3 (completes the rank-1 fold).
                    # Engine alternates per vlane: even vlanes ACT, odd DVE.
                    bcol = (3, 4) if t == 0 else (0, 1)
                    for v in range(NL):
                        if v % 2 == 0:
                            nc.scalar.activation(
                                h[v][:, :], P[v][:, :],
                                mybir.ActivationFunctionType.Relu,
                                bias=bp[:, bcol[0] : bcol[1]], scale=1.0,
                            )
                        else:
                            nc.vector.tensor_scalar(
                                h[v][:, :], P[v][:, :],
                                bp[:, bcol[0] : bcol[1]], 0.0,
                                mybir.AluOpType.add, mybir.AluOpType.max,
                            )
                    # ---- layer 2 (rotated like layer 1) ----
                    for pair in range(G // 2):
                        for v in range(NL):
                            for par in range(2):
                                c = 2 * pair + par
                                blk = pair * CH
                                nc.tensor.matmul(
                                    P[v][pH2(c) : pH2(c) + HID, blk : blk + CH],
                                    wp[pH(c, v) : pH(c, v) + HID, W2_OFF : W2_OFF + HID],
                                    h[v][pH(c, v) : pH(c, v) + HID, blk : blk + CH],
                                    start=True,
                                    stop=True,
                                    tile_position=(pH(c, v), pH2(c)),
                                )
                    # ---- act2: h2 = relu(pre2 + b2); odd vlanes ACT ----
                    for v in range(NL):
                        if v % 2 == 1:
                            nc.scalar.activation(
                                h2[v][:, :], P[v][:, :],
                                mybir.ActivationFunctionType.Relu,
                                bias=bp[:, 1:2], scale=1.0,
                            )
                        else:
                            nc.vector.tensor_scalar(
                                h2[v][:, :], P[v][:, :],
                                bp[:, 1:2], 0.0,
                                mybir.AluOpType.add, mybir.AluOpType.max,
                            )
                    # ---- layer 3: K=128 pair-matmuls -> delta rows ----
                    # one MM per chunk pair computes both parities' deltas
                    # (lhsT block i: W3 at rows 0:64 col i / rows 64:128
                    # col 16+i); one accumulation group per vlane in bank
                    # DBANK at col-group 32*v, rotated for overlap
                    dblk = DBANK * CH
                    for i in range(G // 2):
                        for v in range(NL):
                            dr = 32 * (v % 2)
                            nc.tensor.matmul(
                                P[v][dr : dr + 32, dblk : dblk + CH],
                                wp[0:128, W3_OFF + 32 * i : W3_OFF + 32 * (i + 1)],
                                h2[v][0:128, i * CH : (i + 1) * CH],
                                start=(i == 0),
                                stop=(i == G // 2 - 1),
                                tile_position=(0, dr),
                            )
                    # ---- act3: delta = deltapre + b3; vlanes 0,1,2 ACT ----
                    # even chunks at rows dr:dr+2, odd at dr+16:dr+18
                    for v, p in enumerate(lanes):
                        dr = 32 * (v % 2)
                        if v < 3:
                            nc.scalar.activation(
                                d_new[v][0:18, :], P[v][dr : dr + 18, dblk : dblk + CH],
                                mybir.ActivationFunctionType.Copy,
                                bias=float(b3val), scale=1.0,
                            )
                        else:
                            nc.vector.tensor_scalar(
                                d_new[v][0:18, :], P[v][dr : dr + 18, dblk : dblk + CH],
                                float(b3val), None, mybir.AluOpType.add,
                            )
                        for par in range(2):
                            nc.sync.dma_start(
                                out=out_p[t, par, p * (G // 2) : (p + 1) * (G // 2), :],
                                in_=d_new[v][16 * par : 16 * par + G // 2, :],
                            )
                        h2prev[v] = h2[v]
    return nc


LAST_RESULT = None


def kernel(**inputs):
    return _run(inputs, NPASS, T)


def _prepare(inputs, npass, nsteps):
    features = np.asarray(inputs["features"], dtype=np.float32)
    W1 = np.asarray(inputs["W1"], dtype=np.float32)
    b1 = np.asarray(inputs["b1"], dtype=np.float32)
    W2 = np.asarray(inputs["W2"], dtype=np.float32)
    b2 = np.asarray(inputs["b2"], dtype=np.float32)
    W3 = np.asarray(inputs["W3"], dtype=np.float32)
    b3 = np.asarray(inputs["b3"], dtype=np.float32)

    nc = _build_graph(npass, nsteps, float(b3[0]))
    nc.finalize()

    # host-side packing
    bf = ml_dtypes.bfloat16
    wpack = np.zeros((128, WCOLS), np.float32)
    M = W3 @ W1[3:4]  # (64, 64) rank-1: M[i, j] = W3[i] * W1[3, j]
    for half in (0, 64):
        wpack[half : half + 3, W1A_OFF : W1A_OFF + HID] = W1[0:3]
        wpack[half : half + HID, M_OFF : M_OFF + HID] = M
        wpack[half : half + HID, W2_OFF : W2_OFF + HID] = W2
    for i in range(G // 2):
        wpack[0:HID, W3_OFF + 32 * i + i] = W3[:, 0]
        wpack[64 : 64 + HID, W3_OFF + 32 * i + 16 + i] = W3[:, 0]
    wpack = wpack.astype(bf)

    b1p = b1 + W1[3] * b3[0]
    biasp = np.zeros((128, 4), np.float32)
    for half in (0, 64):
        biasp[half : half + HID, 0] = b1p
        biasp[half : half + HID, 1] = b2
        biasp[half : half + HID, 3] = b1
    biasp[:, 2] = b3[0]

    in_maps = []
    for k in range(NCORES):
        sh = features[k * NC : (k + 1) * NC]          # (NC, T, 3)
        feats = np.ascontiguousarray(sh.transpose(1, 2, 0)).astype(bf)  # (T,3,NC)
        in_maps.append({"feats": feats, "wpack": wpack, "biasp": biasp})

    return nc, in_maps


def _run(inputs, npass, nsteps, trace=False):
    global LAST_RESULT
    from concourse.bass_utils import run_bass_kernel_spmd

    nc, in_maps = _prepare(inputs, npass, nsteps)
    res = run_bass_kernel_spmd(
        nc, in_maps, core_ids=list(range(NCORES)), trace=trace
    )
    LAST_RESULT = res
    outs = res.results

    full = np.empty((N_TOT, T), np.float32)
    for k in range(NCORES):
        o = np.asarray(outs[k]["out"]).astype(np.float32)  # (T, 2, 32, CH)
        # o[t, par, p*(G//2) + j, :] holds chunk c=2j+par of pass p
        o = o.reshape(T, 2, NPASS, G // 2, CH).transpose(0, 2, 3, 1, 4)
        full[k * NC : (k + 1) * NC, :] = o.reshape(T, NC).T
    return full


if __name__ == "__main__":
    import reference

    inputs = reference.setup_inputs()
    out = kernel(**{k: np.asarray(v) for k, v in inputs.items()})
    print("kernel out", out.shape, out.dtype)
